# revision 3
# baseline (speedup 1.0000x reference)
"""GQA attention (B=2, S=2048, D=4096, 32 q-heads, 8 kv-heads) on 8 trn2
NeuronCores.

Strategy (tensor-parallel attention + token-parallel output projection):
  - core c gets wq[:, 512c:512(c+1)] (4 q-heads), wk/wv[:, 128c:128(c+1)]
    (1 kv-head), full x and full wo.
  - per core: PE-transpose x into x^T tiles (channels on partitions),
    project Q^T/K^T (head dim on partitions) and V, run attention for its
    4 heads over all tokens in the scores-transposed layout
    (S^T[k,q] tiles; softmax denominator via a ones-column matmul on the
    PE; no row-max subtraction — |scores| < ~10 so exp is safe in fp32),
  - one AllToAll flips head-sharding into token-sharding, then each core
    computes out[tokens_c, :] = attn^T.T @ wo with no cross-core
    reduction. Host concatenates the 8 token slices.
  All matmuls run in float32r (TF32-like: 1+8+11 bits, full PE rate).
"""
import numpy as np

import concourse.bass as bass
import concourse.mybir as mybir
import concourse.tile as tile
from concourse.bass_utils import run_bass_kernel_spmd

F32 = mybir.dt.float32
F32R = mybir.dt.float32r
BF16 = mybir.dt.bfloat16
AF = mybir.ActivationFunctionType
OP = mybir.AluOpType

P = 128
B, S, D = 2, 2048, 4096
NH, NKV, HD = 32, 8, 128
NCORES = 8
QH = NH // NCORES            # 4 q-heads per core
DQ = QH * HD                 # 512
TOK = B * S                  # 4096
TSLICE = TOK // NCORES       # 512 tokens per core for the wo phase
CT = D // P                  # 32 channel tiles
TCH = 256                    # phase-1 token chunk
NCH = S // TCH               # 8 chunks per batch
KTB = S // P                 # 16 key tiles per batch
QC = 512                     # attention query chunk
NQC = S // QC                # 4 per batch
SCALE = 1.0 / float(np.sqrt(HD))

# ---------------------------------------------------------------------------
# workarounds for this walrus build (max ~1 sync wait per instruction)
# ---------------------------------------------------------------------------

def _patched_drain_and_barrier(self, tick_clock, wait_clock):
    from concourse.vector_clock import ScopedClock

    nop_inst = self.nc.sync.nop(nofuse=True, hint="drain_waits")
    wait_clock.add_sem_waits(
        nop_inst.ins, ScopedClock({None: tick_clock.global_clock})
    )
    si = nop_inst.ins.sync_info
    waits = list(si.on_wait or [])
    if len(waits) > 1:
        si.on_wait = waits[:1]
        for i in range(1, len(waits)):
            extra = self.nc.sync.nop(nofuse=True, hint="drain_waits")
            extra.ins.sync_info = mybir.SyncInfo(on_wait=[waits[i]], on_update=[])
    self.nc.sync.drain()
    self.nc.all_engine_barrier()
    assert self.sems is not None
    popped = self.nc._tile_sem_poison_stack.pop()
    assert popped is self._sem_poison
    self.nc.clear_and_free_semaphores(list(self.sems.allocated().values()))
    self.nc.all_engine_barrier()


def _install_tile_patch():
    tile.TileContext._drain_and_barrier = _patched_drain_and_barrier


def _legalize_waits(nc, max_waits=1):
    n_split = 0
    for bb in nc.main_func.blocks:
        insts = bb.instructions
        new_list = []
        changed = False
        for inst in insts:
            si = inst.sync_info
            waits = list(si.on_wait) if si and si.on_wait else []
            if len(waits) > max_waits:
                keep = waits[-max_waits:]
                extra = waits[: len(waits) - max_waits]
                for i in range(0, len(extra), max_waits):
                    chunk = extra[i : i + max_waits]
                    nop = mybir.InstNoOp(
                        name=nc.get_next_instruction_name(),
                        engine=inst.engine,
                        sync_info=mybir.SyncInfo(on_wait=chunk, on_update=[]),
                        text_hint="wait_split",
                        bass_nofuse=True,
                    )
                    nc.register_instruction(nop)
                    new_list.append(nop)
                inst.sync_info = mybir.SyncInfo(
                    on_wait=keep, on_update=list(si.on_update or [])
                )
                n_split += 1
                changed = True
            new_list.append(inst)
        if changed:
            bb.instructions = new_list
    return n_split


# ---------------------------------------------------------------------------
# host-side fp32r rounding (1+8+11-bit, round to nearest even)
# ---------------------------------------------------------------------------

def _round_f32r(a):
    bits = np.ascontiguousarray(a, dtype=np.float32).view(np.uint32).astype(np.uint64)
    lsb = (bits >> 12) & 1
    bits = (((bits + 2047 + lsb) >> 12) << 12) & 0xFFFFFFFF
    return bits.astype(np.uint32).view(np.float32)


# ---------------------------------------------------------------------------
# kernel build
# ---------------------------------------------------------------------------

def _build_nc(reps=1, sim=False):
    nc = bass.Bass()
    x = nc.declare_dram_parameter("x", [TOK, D], F32R, isOutput=False)
    wq = nc.declare_dram_parameter("wq", [D, DQ], F32R, isOutput=False)
    wk = nc.declare_dram_parameter("wk", [D, HD], F32R, isOutput=False)
    wv = nc.declare_dram_parameter("wv", [D, HD], F32R, isOutput=False)
    wo = nc.declare_dram_parameter("wo", [D, D], F32R, isOutput=False)
    ident = nc.declare_dram_parameter("ident", [P, P], F32R, isOutput=False)
    out = nc.declare_dram_parameter("out", [TSLICE, D], F32, isOutput=True)

    x3 = x.rearrange("(tt p) d -> tt p d", p=P)          # [32, 128, 4096]
    wq3 = wq.rearrange("(ct p) m -> p ct m", p=P)        # [128, 32, 512]
    wk3 = wk.rearrange("(ct p) m -> p ct m", p=P)        # [128, 32, 128]
    wv3 = wv.rearrange("(ct p) m -> p ct m", p=P)        # [128, 32, 128]
    wo3 = wo.rearrange("(ht p) e -> p ht e", p=P)        # [128, 32, 4096]
    out3 = out.rearrange("(tt p) e -> p tt e", p=P)      # [128, 4, 4096]

    with tile.TileContext(nc) as tc:
        with (
            tc.tile_pool(name="consts", bufs=1) as consts,
            tc.tile_pool(name="dram", bufs=1, space="DRAM") as dram,
        ):
            identity = consts.tile([P, P], F32R)
            nc.sync.dma_start(identity[:], ident[:])
            ones_f = consts.tile([P, 1], F32)
            nc.gpsimd.memset(ones_f[:], 1.0)
            ones_col = consts.tile([P, 1], F32R)
            nc.vector.tensor_copy(ones_col[:], ones_f[:])
            ones_rf = consts.tile([1, P], F32)
            nc.gpsimd.memset(ones_rf[:], 1.0)
            ones_row = consts.tile([1, P], F32R)
            nc.vector.tensor_copy(ones_row[:], ones_rf[:])

            a2a_in_lo = dram.tile([NCORES, 2 * HD, TSLICE], F32R)
            a2a_in_hi = dram.tile([NCORES, 2 * HD, TSLICE], F32R)
            a2a_out_lo = dram.tile([NCORES, 2 * HD, TSLICE], F32R)
            a2a_out_hi = dram.tile([NCORES, 2 * HD, TSLICE], F32R)

            for rep in range(reps):
              with (
                  tc.tile_pool(name="wts", bufs=1) as wts,
                  tc.tile_pool(name="batch", bufs=1) as batch,
                  tc.tile_pool(name="xs", bufs=2) as xsp,
                  tc.tile_pool(name="xts", bufs=1) as xtsp,
                  tc.tile_pool(name="expp", bufs=2) as expp,
                  tc.tile_pool(name="aop", bufs=1) as aop,
                  tc.tile_pool(name="recp", bufs=1) as recp,
                  tc.tile_pool(name="qnp", bufs=2) as qnp,
                  tc.tile_pool(name="psA", bufs=2, space="PSUM") as psA,
                  tc.tile_pool(name="psB", bufs=1, space="PSUM") as psB,
              ):
                wq_sb = wts.tile([P, CT, DQ], F32R)
                nc.scalar.dma_start(wq_sb[:], wq3[:])
                wkv_sb = wts.tile([P, CT, 2 * HD], F32R)
                nc.scalar.dma_start(wkv_sb[:, :, 0:HD], wk3[:])
                nc.scalar.dma_start(wkv_sb[:, :, HD:2 * HD], wv3[:])
                for b in range(B):
                    qt_sb = batch.tile([P, QH, S], F32R, tag="qt")
                    kt_sb = batch.tile([P, S], F32R, tag="kt")
                    kv_sb = batch.tile([P, KTB, 2 * HD], F32R, tag="kv")

                    # ---- phase 1: transpose x chunk + QKV projections ----
                    for ch in range(NCH):
                        xts_t = xtsp.tile([P, CT, TCH], F32R, tag="xts")
                        for i in range(2):  # two 128-token tiles per chunk
                            tt = b * (S // P) + ch * 2 + i
                            for eighth in range(8):
                                xst = xsp.tile([P, D // 8], F32R, tag="xs")
                                nc.sync.dma_start(
                                    xst[:], x3[tt, :, eighth * (D // 8):(eighth + 1) * (D // 8)]
                                )
                                pst = psA.tile([P, 4, P], F32R, tag="big")
                                for u in range(4):
                                    nc.tensor.matmul(
                                        pst[:, u, :],
                                        xst[:, u * P:(u + 1) * P],
                                        identity[:],
                                        is_transpose=True,
                                        skip_group_check=(u > 0),
                                    )
                                ct0 = eighth * 4
                                nc.vector.tensor_copy(
                                    xts_t[:, ct0:ct0 + 4, i * P:(i + 1) * P], pst[:]
                                )
                        for tsub in range(2):
                            kt_idx = ch * 2 + tsub
                            xsl = slice(tsub * P, (tsub + 1) * P)
                            psq = psA.tile([P, DQ], F32, tag="med")
                            pskv = psA.tile([P, 2 * HD], F32, tag="med")
                            for ct in range(CT):
                                nc.tensor.matmul(
                                    psq[:], xts_t[:, ct, xsl], wq_sb[:, ct, :],
                                    start=(ct == 0), stop=(ct == CT - 1),
                                )
                                nc.tensor.matmul(
                                    pskv[:], xts_t[:, ct, xsl], wkv_sb[:, ct, :],
                                    start=(ct == 0), stop=(ct == CT - 1),
                                )
                            qn = qnp.tile([P, DQ], F32R, tag="qn")
                            nc.vector.tensor_copy(qn[:], psq[:])
                            nc.vector.tensor_copy(kv_sb[:, kt_idx, :], pskv[:])
                            pst = psA.tile([P, 4, P], F32R, tag="big")
                            for hd in range(QH):
                                nc.tensor.matmul(
                                    pst[:, hd, :],
                                    qn[:, hd * P:(hd + 1) * P],
                                    identity[:],
                                    is_transpose=True,
                                    skip_group_check=(hd > 0),
                                )
                            nc.vector.tensor_copy(
                                qt_sb[:, 0:QH, kt_idx * P:(kt_idx + 1) * P], pst[:]
                            )
                            pskt = psB.tile([P, HD], F32R, tag="sm")
                            nc.tensor.matmul(
                                pskt[:], kv_sb[:, kt_idx, 0:HD], identity[:],
                                is_transpose=True,
                            )
                            nc.vector.tensor_copy(
                                kt_sb[:, kt_idx * P:(kt_idx + 1) * P], pskt[:]
                            )

                    # ---- phase 2: attention for this batch ----
                    for qc in range(NQC):
                        j = b * NQC + qc     # destination core for these tokens
                        qsl = slice(qc * QC, (qc + 1) * QC)
                        for h in range(QH):
                            numer = psA.tile([P, QC], F32, tag="med")
                            den4 = psB.tile([P, QC], F32, tag="dn")
                            for kp in range(KTB // 2):
                                pss = psA.tile([P, 2, QC], F32, tag="big")
                                for u in range(2):
                                    kt = kp * 2 + u
                                    nc.tensor.matmul(
                                        pss[:, u, :],
                                        kt_sb[:, kt * P:(kt + 1) * P],
                                        qt_sb[:, h, qsl],
                                        start=True, stop=True,
                                        skip_group_check=(u > 0),
                                    )
                                et = expp.tile([P, 2, QC], F32R, tag="exp")
                                nc.scalar.activation(et[:], pss[:], AF.Exp, scale=SCALE)
                                for u in range(2):
                                    kt = kp * 2 + u
                                    first = kp == 0 and u == 0
                                    last = kp == KTB // 2 - 1 and u == 1
                                    nc.tensor.matmul(
                                        numer[:], kv_sb[:, kt, HD:2 * HD], et[:, u, :],
                                        start=first, stop=last,
                                    )
                                    nc.tensor.matmul(
                                        den4[0:1, :], ones_col[:], et[:, u, :],
                                        start=first, stop=last,
                                    )
                            rec = recp.tile([1, QC], F32R, tag="rec")
                            with nc.allow_low_precision(reason="softmax recip in f32r"):
                                nc.vector.reciprocal(rec[:], den4[0:1, :])
                            rbc = psA.tile([P, QC], F32, tag="big")
                            nc.tensor.matmul(
                                rbc[:], ones_row[:], rec[:], start=True, stop=True
                            )
                            rbs = qnp.tile([P, QC], F32R, tag="qn")
                            nc.vector.tensor_copy(rbs[:], rbc[:])
                            ao = aop.tile([P, QC], F32R, tag="ao")
                            nc.vector.tensor_tensor(
                                ao[:], numer[:], rbs[:], OP.mult
                            )
                            a2a_dst = a2a_in_lo if h < 2 else a2a_in_hi
                            nc.sync.dma_start(
                                a2a_dst[j, (h % 2) * P:(h % 2 + 1) * P, :], ao[:]
                            )

              # ---- AllToAll: head-sharded -> token-sharded (two waves) ----
              if True:
                if sim:
                    # timing-sim stand-in: local DRAM->DRAM copy of the same bytes
                    nc.gpsimd.dma_start(a2a_out_lo[:], a2a_in_lo[:])
                    nc.gpsimd.dma_start(a2a_out_hi[:], a2a_in_hi[:])
                else:
                    nc.gpsimd.collective_compute(
                        "AllToAll",
                        OP.bypass,
                        ins=[a2a_in_lo.opt()],
                        outs=[a2a_out_lo.opt()],
                        replica_groups=[list(range(NCORES))],
                    )
                    nc.gpsimd.collective_compute(
                        "AllToAll",
                        OP.bypass,
                        ins=[a2a_in_hi.opt()],
                        outs=[a2a_out_hi.opt()],
                        replica_groups=[list(range(NCORES))],
                    )

                # ---- phase 3: out[tokens_c, :] = attnT.T @ wo ----
                with (
                    tc.tile_pool(name="wop", bufs=1) as wop,
                    tc.tile_pool(name="wos", bufs=64) as wos,
                    tc.tile_pool(name="outp", bufs=3) as outp,
                    tc.tile_pool(name="ps3", bufs=2, space="PSUM") as ps3,
                ):
                    a2a_sb_lo = wop.tile([P, 16, TSLICE], F32R)
                    nc.sync.dma_start(
                        a2a_sb_lo[:],
                        a2a_out_lo[:].rearrange("j (g2 p) t -> p (j g2) t", p=P),
                    )
                    a2a_sb_hi = wop.tile([P, 16, TSLICE], F32R)
                    nc.sync.dma_start(
                        a2a_sb_hi[:],
                        a2a_out_hi[:].rearrange("j (g2 p) t -> p (j g2) t", p=P),
                    )
                    psos_live = {}

                    def group_a(ec):
                        esl = slice(ec * 512, (ec + 1) * 512)
                        psos_live[ec] = [
                            ps3.tile([P, 512], F32, tag=f"wo{tt}", name=f"pso{ec}_{tt}")
                            for tt in range(TSLICE // P)
                        ]
                        first = True
                        for j in range(NCORES):
                            for hl in range(2):
                                ht = 4 * j + hl
                                wo_t = wos.tile([P, 512], F32R, tag="wo_t", name="wo_a")
                                nc.scalar.dma_start(wo_t[:], wo3[:, ht, esl])
                                for tt in range(TSLICE // P):
                                    nc.tensor.matmul(
                                        psos_live[ec][tt][:],
                                        a2a_sb_lo[:, j * 2 + hl, tt * P:(tt + 1) * P],
                                        wo_t[:],
                                        start=first, stop=False,
                                    )
                                first = False

                    def group_b(ec):
                        esl = slice(ec * 512, (ec + 1) * 512)
                        for j in range(NCORES):
                            for hl in range(2):
                                ht = 4 * j + 2 + hl
                                wo_t = wos.tile([P, 512], F32R, tag="wo_t", name="wo_b")
                                nc.scalar.dma_start(wo_t[:], wo3[:, ht, esl])
                                for tt in range(TSLICE // P):
                                    nc.tensor.matmul(
                                        psos_live[ec][tt][:],
                                        a2a_sb_hi[:, j * 2 + hl, tt * P:(tt + 1) * P],
                                        wo_t[:],
                                        start=False,
                                        stop=(j == NCORES - 1 and hl == 1),
                                    )
                        for tt in range(TSLICE // P):
                            ot = outp.tile([P, 512], F32, tag="ot", name="ot")
                            nc.vector.tensor_copy(ot[:], psos_live[ec][tt][:])
                            nc.sync.dma_start(out3[:, tt, esl], ot[:])
                        del psos_live[ec]

                    group_a(0)
                    for ec in range(1, 8):
                        group_a(ec)
                        group_b(ec - 1)
                    group_b(7)

    _legalize_waits(nc)
    return nc


_NC_CACHE = {}


def _get_nc(reps=1):
    if reps not in _NC_CACHE:
        _install_tile_patch()
        _NC_CACHE[reps] = _build_nc(reps)
    return _NC_CACHE[reps]


def make_in_maps(x, wq, wk, wv, wo):
    xf = _round_f32r(np.asarray(x, dtype=np.float32).reshape(TOK, D))
    wqf = _round_f32r(wq)
    wkf = _round_f32r(wk)
    wvf = _round_f32r(wv)
    wof = _round_f32r(wo)
    identv = np.eye(P, dtype=np.float32)
    in_maps = []
    for c in range(NCORES):
        in_maps.append({
            "x": xf,
            "ident": identv,
            "wq": np.ascontiguousarray(wqf[:, c * DQ:(c + 1) * DQ]),
            "wk": np.ascontiguousarray(wkf[:, c * HD:(c + 1) * HD]),
            "wv": np.ascontiguousarray(wvf[:, c * HD:(c + 1) * HD]),
            "wo": wof,
        })
    return in_maps


def assemble_output(results):
    out = np.concatenate([results[c]["out"] for c in range(NCORES)], axis=0)
    return out.reshape(B, S, D)


def kernel(x, wq, wk, wv, wo):
    nc = _get_nc(reps=1)
    in_maps = make_in_maps(x, wq, wk, wv, wo)
    res = run_bass_kernel_spmd(nc, in_maps, list(range(NCORES)))
    return assemble_output(res.results)


if __name__ == "__main__":
    rng = np.random.default_rng(0)
    xv = rng.standard_normal((B, S, D), dtype=np.float32)
    wqv = (rng.standard_normal((D, NH * HD), dtype=np.float32) * 0.02)
    wkv = (rng.standard_normal((D, NKV * HD), dtype=np.float32) * 0.02)
    wvv = (rng.standard_normal((D, NKV * HD), dtype=np.float32) * 0.02)
    wov = (rng.standard_normal((NH * HD, D), dtype=np.float32) * 0.02)
    got = kernel(xv, wqv, wkv, wvv, wov)
    print("kernel output", got.shape, got.dtype)



# revision 47
# speedup vs baseline: 11.4220x; 11.4220x over previous
"""GQA attention (B=2, S=2048, D=4096, 32 q-heads, 8 kv-heads) on 8 trn2
NeuronCores.

Tensor-parallel attention + token-parallel output projection, bf16 compute
with fp32 PSUM accumulation:
  - core c gets wq[:, 512c:512(c+1)] (4 q-heads), wk/wv[:, 128c:128(c+1)]
    (1 kv-head), full x and full wo; everything cast to bf16 on host.
  - phase 1: x^T tiles come from DMA XBAR transposes (no PE transposes);
    Q^T is projected directly (wq chunks stationary, x^T moving), K/V
    projected in natural layout and K^T derived by SBUF->SBUF DMA
    transpose.
  - phase 2 (head-outer, software-pipelined): scores S^T = K^T.T @ Q^T per
    128-key tile, exp on the scalar engine (scale folded in, no row-max:
    |scores|<~10), denominator = DVE bf16 tile-accumulate + one ones-column
    matmul, numer = V^T.T @ exp accumulated in PSUM. One AllToAll wave per
    query head flips head-sharding into token-sharding as soon as that head
    finishes, hiding the collective under the next head's compute.
  - phase 3: out[tokens_c, :] = attn^T.T @ wo, no cross-core reduction.
    Host concatenates the 8 token slices.
"""
import numpy as np
import ml_dtypes

import concourse.bass as bass
import concourse.mybir as mybir
import concourse.tile as tile
from concourse.bass_utils import run_bass_kernel_spmd

F32 = mybir.dt.float32
BF16 = mybir.dt.bfloat16
AF = mybir.ActivationFunctionType
OP = mybir.AluOpType

P = 128
B, S, D = 2, 2048, 4096
NH, NKV, HD = 32, 8, 128
NCORES = 8
QH = NH // NCORES            # 4 q-heads per core
DQ = QH * HD                 # 512
TOK = B * S                  # 4096
TSLICE = TOK // NCORES       # 512 tokens per core for the wo phase
CT = D // P                  # 32 channel tiles
TCH = 512                    # phase-1 token chunk
NCH = TOK // TCH             # 8 chunks over both batches
KTB = S // P                 # 16 key tiles per batch
QC = 512                     # attention query chunk
NQC = S // QC                # 4 per batch
SCALE = 1.0 / float(np.sqrt(HD))

# ---------------------------------------------------------------------------
# workarounds for this walrus build (max ~1 sync wait per instruction)
# ---------------------------------------------------------------------------

def _patched_drain_and_barrier(self, tick_clock, wait_clock):
    from concourse.vector_clock import ScopedClock

    nop_inst = self.nc.sync.nop(nofuse=True, hint="drain_waits")
    wait_clock.add_sem_waits(
        nop_inst.ins, ScopedClock({None: tick_clock.global_clock})
    )
    si = nop_inst.ins.sync_info
    waits = list(si.on_wait or [])
    if len(waits) > 1:
        si.on_wait = waits[:1]
        for i in range(1, len(waits)):
            extra = self.nc.sync.nop(nofuse=True, hint="drain_waits")
            extra.ins.sync_info = mybir.SyncInfo(on_wait=[waits[i]], on_update=[])
    self.nc.sync.drain()
    self.nc.all_engine_barrier()
    assert self.sems is not None
    popped = self.nc._tile_sem_poison_stack.pop()
    assert popped is self._sem_poison
    self.nc.clear_and_free_semaphores(list(self.sems.allocated().values()))
    self.nc.all_engine_barrier()


def _install_tile_patch():
    tile.TileContext._drain_and_barrier = _patched_drain_and_barrier


def _legalize_waits(nc, max_waits=1):
    n_split = 0
    for bb in nc.main_func.blocks:
        insts = bb.instructions
        new_list = []
        changed = False
        for inst in insts:
            si = inst.sync_info
            waits = list(si.on_wait) if si and si.on_wait else []
            if len(waits) > max_waits:
                keep = waits[-max_waits:]
                extra = waits[: len(waits) - max_waits]
                for i in range(0, len(extra), max_waits):
                    chunk = extra[i : i + max_waits]
                    nop = mybir.InstNoOp(
                        name=nc.get_next_instruction_name(),
                        engine=inst.engine,
                        sync_info=mybir.SyncInfo(on_wait=chunk, on_update=[]),
                        text_hint="wait_split",
                        bass_nofuse=True,
                    )
                    nc.register_instruction(nop)
                    new_list.append(nop)
                inst.sync_info = mybir.SyncInfo(
                    on_wait=keep, on_update=list(si.on_update or [])
                )
                n_split += 1
                changed = True
            new_list.append(inst)
        if changed:
            bb.instructions = new_list
    return n_split


# ---------------------------------------------------------------------------
# kernel build
# ---------------------------------------------------------------------------

def _build_nc(reps=1, sim=False, debug=False):
    nc = bass.Bass()
    x = nc.declare_dram_parameter("x", [TOK, D], BF16, isOutput=False)
    wq = nc.declare_dram_parameter("wq", [D, DQ], BF16, isOutput=False)
    wk = nc.declare_dram_parameter("wk", [D, HD], BF16, isOutput=False)
    wv = nc.declare_dram_parameter("wv", [D, HD], BF16, isOutput=False)
    wo = nc.declare_dram_parameter("wo", [D, D], BF16, isOutput=False)
    out = nc.declare_dram_parameter("out", [TSLICE, D], F32, isOutput=True)
    if debug:
        dbg_qt0 = nc.declare_dram_parameter("dbg_qt0", [P, QH, S], BF16, isOutput=True)
        dbg_kt0 = nc.declare_dram_parameter("dbg_kt0", [P, S], BF16, isOutput=True)
        dbg_kv0 = nc.declare_dram_parameter("dbg_kv0", [P, KTB, 2 * HD], BF16, isOutput=True)
        dbg_a2ain = nc.declare_dram_parameter("dbg_a2ain", [QH, NCORES, HD, TSLICE], BF16, isOutput=True)
        dbg_a2asb = nc.declare_dram_parameter("dbg_a2asb", [QH, P, NCORES, TSLICE], BF16, isOutput=True)

    ident = nc.declare_dram_parameter("ident", [P, P], BF16, isOutput=False)
    x3 = x.rearrange("(tt p) d -> tt p d", p=P)          # [32, 128, 4096]
    wq3 = wq.rearrange("(ct p) m -> p ct m", p=P)        # [128, 32, 512]
    wk3 = wk.rearrange("(ct p) m -> p ct m", p=P)        # [128, 32, 128]
    wv3 = wv.rearrange("(ct p) m -> p ct m", p=P)        # [128, 32, 128]
    wo3 = wo.rearrange("(ht p) e -> p ht e", p=P)        # [128, 32, 4096]
    wo4 = wo.rearrange("(j four p) e -> p j four e", p=P, four=4)  # [128,8,4,4096]
    out3 = out.rearrange("(tt p) e -> p tt e", p=P)      # [128, 4, 4096]

    with tile.TileContext(nc) as tc:
        with (
            tc.tile_pool(name="consts", bufs=1) as consts,
            tc.tile_pool(name="dram", bufs=1, space="DRAM") as dram,
        ):
            identity = consts.tile([P, P], BF16)
            nc.sync.dma_start(identity[:], ident[:])
            ones_f = consts.tile([P, 1], F32)
            nc.gpsimd.memset(ones_f[:], 1.0)
            ones_col = consts.tile([P, 1], BF16)
            nc.vector.tensor_copy(ones_col[:], ones_f[:])
            ones_rf = consts.tile([1, P], F32)
            nc.gpsimd.memset(ones_rf[:], 1.0)
            ones_row = consts.tile([1, P], BF16)
            nc.vector.tensor_copy(ones_row[:], ones_rf[:])

            # one AllToAll wave per local q-head: [dest core, hd, tokens]
            a2a_in = [dram.tile([NCORES, HD, TSLICE], BF16, name=f"a2a_in{h}")
                      for h in range(QH)]
            a2a_out = [dram.tile([NCORES, HD, TSLICE], BF16, name=f"a2a_out{h}")
                       for h in range(QH)]

            for rep in range(reps):
              with tc.tile_pool(name="a2asb", bufs=1) as a2asbp:
               with (
                  tc.tile_pool(name="qkv", bufs=1) as qkv,
               ):
                qt_sb = [qkv.tile([P, QH, S], BF16, name=f"qt{b}") for b in range(B)]
                kt_sb = [qkv.tile([P, S], BF16, name=f"kt{b}") for b in range(B)]
                kv_sb = [qkv.tile([P, KTB, 2 * HD], BF16, name=f"kv{b}")
                         for b in range(B)]

                # ---- phase 1: transpose x on PE, project Q^T / K / V ----
                with (
                    tc.tile_pool(name="wts", bufs=1) as wts,
                    tc.tile_pool(name="xts", bufs=2) as xtsp,
                    tc.tile_pool(name="xsrc", bufs=2) as xsrcp,
                    tc.tile_pool(name="psq", bufs=1, space="PSUM") as psq,
                    tc.tile_pool(name="pskv", bufs=1, space="PSUM") as pskv,
                    tc.tile_pool(name="pst", bufs=2, space="PSUM") as pstp,
                    tc.tile_pool(name="pkt", bufs=1, space="PSUM") as pktp,
                ):
                    wq_sb = wts.tile([P, CT, DQ], BF16)
                    wkv_sb = wts.tile([P, CT, 2 * HD], BF16)
                    first = True
                    for ch in range(NCH):
                        b = ch // NQC
                        qc = ch % NQC
                        xts = xtsp.tile([P, CT, TCH], BF16, tag="xts")
                        for tt in range(TCH // P):
                          for th in range(4):
                            xsrc = xsrcp.tile([P, D // 4], BF16, tag="xsrc")
                            nc.sync.dma_start(
                                xsrc[:],
                                x3[ch * (TCH // P) + tt, :,
                                   th * (D // 4):(th + 1) * (D // 4)],
                            )
                            for cg in range(CT // 16):
                                pst = pstp.tile([P, 4, P], BF16, tag="t")
                                for u in range(4):
                                    ci = cg * 4 + u
                                    nc.tensor.matmul(
                                        pst[:, u, :],
                                        xsrc[:, ci * P:(ci + 1) * P],
                                        identity[:],
                                        is_transpose=True,
                                        skip_group_check=(u > 0),
                                    )
                                ct0 = th * (CT // 4) + cg * 4
                                nc.vector.tensor_copy(
                                    xts[:, ct0:ct0 + 4, tt * P:(tt + 1) * P],
                                    pst[:],
                                )
                        if first:
                            # weights after chunk 0's transposes, split by
                            # what compute consumes first (per q-head chunk)
                            nc.scalar.dma_start(
                                wq_sb[:, :, 0:P], wq3[:, :, 0:P]
                            )
                            nc.scalar.dma_start(wkv_sb[:, :, 0:HD], wk3[:])
                            nc.scalar.dma_start(wkv_sb[:, :, HD:2 * HD], wv3[:])
                            for h in range(1, QH):
                                nc.scalar.dma_start(
                                    wq_sb[:, :, h * P:(h + 1) * P],
                                    wq3[:, :, h * P:(h + 1) * P],
                                )
                            first = False
                        # ct4-outer so chunk-0 compute streams at DMA pace
                        pqs = [psq.tile([P, TCH], F32, tag=f"q{h}", name=f"pq{h}")
                               for h in range(QH)]
                        for ct4 in range(CT // 4):
                            for h in range(QH):
                                for ci in range(4):
                                    ct = ct4 * 4 + ci
                                    nc.tensor.matmul(
                                        pqs[h][:],
                                        wq_sb[:, ct, h * P:(h + 1) * P],
                                        xts[:, ct, :],
                                        start=(ct == 0), stop=(ct == CT - 1),
                                    )
                        for h in range(QH):
                            nc.vector.tensor_copy(
                                qt_sb[b][:, h, qc * QC:(qc + 1) * QC], pqs[h][:]
                            )
                        for tsub in range(TCH // P):
                            kt_idx = qc * (TCH // P) + tsub
                            pkv = pskv.tile([P, 2 * HD], F32, tag="kv")
                            for ct in range(CT):
                                nc.tensor.matmul(
                                    pkv[:],
                                    xts[:, ct, tsub * P:(tsub + 1) * P],
                                    wkv_sb[:, ct, :],
                                    start=(ct == 0), stop=(ct == CT - 1),
                                )
                            nc.vector.tensor_copy(kv_sb[b][:, kt_idx, :], pkv[:])
                            pkt = pktp.tile([P, P], BF16, tag="kt")
                            nc.tensor.matmul(
                                pkt[:], kv_sb[b][:, kt_idx, 0:HD], identity[:],
                                is_transpose=True,
                            )
                            nc.vector.tensor_copy(
                                kt_sb[b][:, kt_idx * P:(kt_idx + 1) * P], pkt[:]
                            )

                if debug and rep == 0:
                    nc.sync.dma_start(dbg_qt0[:], qt_sb[0][:])
                    nc.sync.dma_start(dbg_kt0[:], kt_sb[0][:])
                    nc.sync.dma_start(dbg_kv0[:], kv_sb[0][:])

                # allocated after phase 1 so it reuses the freed xts space
                a2a_sb = [a2asbp.tile([P, NCORES, TSLICE], BF16, name=f"a2asb{h}")
                          for h in range(QH)]

                # ---- phase 2: attention, head-outer, software-pipelined ----
                with (
                    tc.tile_pool(name="pss", bufs=2, space="PSUM") as pssp,
                    tc.tile_pool(name="pnm", bufs=2, space="PSUM") as pnmp,
                    tc.tile_pool(name="pdn", bufs=2, space="PSUM") as pdnp,
                    tc.tile_pool(name="expp", bufs=4) as expp,
                    tc.tile_pool(name="accp", bufs=2) as accp,
                    tc.tile_pool(name="aop", bufs=2) as aop,
                    tc.tile_pool(name="recp", bufs=2) as recp,
                ):
                    items = [(h, b, qc)
                             for h in range(QH)
                             for b in range(B)
                             for qc in range(NQC)]

                    def emit_scores(item, kp):
                        h, b, qc = item
                        qsl = slice(qc * QC, (qc + 1) * QC)
                        pss = pssp.tile([P, 2, QC], F32, tag="s")
                        for u in range(2):
                            kt = kp * 2 + u
                            nc.tensor.matmul(
                                pss[:, u, :],
                                kt_sb[b][:, kt * P:(kt + 1) * P],
                                qt_sb[b][:, h, qsl],
                                start=True, stop=True,
                                skip_group_check=(u > 0),
                            )
                        return pss

                    def stage_wave(h):
                        # pool queue: only collectives live there, so the
                        # collective-wait can't head-block compute queues
                        nc.gpsimd.dma_start(
                            a2a_sb[h][:],
                            a2a_out[h][:].rearrange("j p t -> p j t", p=P),
                        )

                    def emit_wave(h):
                        # stage the PREVIOUS wave first: its collective is
                        # long done, so the pool queue isn't head-blocked and
                        # this wave's collective launches right behind it
                        if h > 0:
                            stage_wave(h - 1)
                        if sim:
                            nc.gpsimd.dma_start(a2a_out[h][:], a2a_in[h][:])
                        else:
                            nc.gpsimd.collective_compute(
                                "AllToAll",
                                OP.bypass,
                                ins=[a2a_in[h].opt()],
                                outs=[a2a_out[h].opt()],
                                replica_groups=[list(range(NCORES))],
                            )

                    pend = None  # deferred finalize: (numer, den, rec, h, j)
                    pss_q = [emit_scores(items[0], 0)]
                    for idx, item in enumerate(items):
                        h, b, qc = item
                        j = b * NQC + qc
                        numer = pnmp.tile([P, QC], F32, tag="n")
                        acc = accp.tile([P, QC], BF16, tag="a")
                        for kp in range(KTB // 2):
                            if kp < KTB // 2 - 1:
                                pss_q.append(emit_scores(item, kp + 1))
                            elif idx + 1 < len(items):
                                pss_q.append(emit_scores(items[idx + 1], 0))
                            cur = pss_q.pop(0)
                            et = expp.tile([P, 2, QC], BF16, tag="e")
                            nc.scalar.activation(et[:], cur[:], AF.Exp, scale=SCALE)
                            if pend is not None:
                                # previous item's tail, placed here so the
                                # PE-side rbc lands in the exp-wait gap
                                p_nm, p_dn, p_rec, p_h, p_j = pend
                                rbc = pdnp.tile([P, QC], F32, tag="d")
                                nc.tensor.matmul(
                                    rbc[:], ones_row[:], p_rec[:],
                                    start=True, stop=True,
                                )
                                rbs = accp.tile([P, QC], BF16, tag="rb")
                                nc.vector.tensor_copy(rbs[:], rbc[:])
                                ao = aop.tile([P, QC], BF16, tag="o")
                                nc.vector.tensor_tensor(
                                    ao[:], p_nm[:], rbs[:], OP.mult
                                )
                                nc.sync.dma_start(a2a_in[p_h][p_j], ao[:])
                                if p_j == NCORES - 1:
                                    emit_wave(p_h)
                                pend = None
                            if kp == 0:
                                nc.vector.tensor_copy(acc[:], et[:, 0, :])
                            else:
                                nc.vector.tensor_tensor(
                                    acc[:], acc[:], et[:, 0, :], OP.add
                                )
                            nc.vector.tensor_tensor(
                                acc[:], acc[:], et[:, 1, :], OP.add
                            )
                            for u in range(2):
                                kt = kp * 2 + u
                                nc.tensor.matmul(
                                    numer[:],
                                    kv_sb[b][:, kt, HD:2 * HD],
                                    et[:, u, :],
                                    start=(kp == 0 and u == 0),
                                    stop=(kp == KTB // 2 - 1 and u == 1),
                                )
                        den = pdnp.tile([P, QC], F32, tag="d")
                        nc.tensor.matmul(
                            den[0:1, :], ones_col[:], acc[:],
                            start=True, stop=True,
                        )
                        rec = recp.tile([1, QC], BF16, tag="r")
                        with nc.allow_low_precision(reason="softmax recip"):
                            nc.vector.reciprocal(rec[:], den[0:1, :])
                        pend = (numer, den, rec, h, j)

                    # flush the last item's tail
                    p_nm, p_dn, p_rec, p_h, p_j = pend
                    rbc = pdnp.tile([P, QC], F32, tag="d")
                    nc.tensor.matmul(
                        rbc[:], ones_row[:], p_rec[:], start=True, stop=True
                    )
                    rbs = accp.tile([P, QC], BF16, tag="rb")
                    nc.vector.tensor_copy(rbs[:], rbc[:])
                    ao = aop.tile([P, QC], BF16, tag="o")
                    nc.vector.tensor_tensor(ao[:], p_nm[:], rbs[:], OP.mult)
                    nc.sync.dma_start(a2a_in[p_h][p_j], ao[:])
                    emit_wave(p_h)
                    stage_wave(QH - 1)

               # ---- phase 3: out[tokens_c, :] = attnT.T @ wo ----
               with (
                  tc.tile_pool(name="wos", bufs=4) as wos,
                  tc.tile_pool(name="outp", bufs=3) as outp,
                  tc.tile_pool(name="ps3", bufs=2, space="PSUM") as ps3,
               ):
                if debug and rep == 0:
                    for h in range(QH):
                        nc.sync.dma_start(dbg_a2ain[h], a2a_in[h][:])
                        nc.sync.dma_start(dbg_a2asb[h], a2a_sb[h][:])
                psos_live = {}

                def group_a(ec):
                    esl = slice(ec * 512, (ec + 1) * 512)
                    psos_live[ec] = [
                        ps3.tile([P, 512], F32, tag=f"wo{tt}", name=f"pso{ec}_{tt}")
                        for tt in range(TSLICE // P)
                    ]
                    # head-tile pairs (hl 0-1) of this ec, one DMA per j
                    wo_t = wos.tile([P, NCORES, 2, 512], BF16, tag="wo_t",
                                    name="wo_a")
                    for j in range(NCORES):
                        nc.sync.dma_start(wo_t[:, j, :, :], wo4[:, j, 0:2, esl])
                    first = True
                    for j in range(NCORES):
                        for hl in range(2):
                            for tt in range(TSLICE // P):
                                nc.tensor.matmul(
                                    psos_live[ec][tt][:],
                                    a2a_sb[hl][:, j, tt * P:(tt + 1) * P],
                                    wo_t[:, j, hl, :],
                                    start=first, stop=False,
                                )
                            first = False

                def group_b(ec):
                    esl = slice(ec * 512, (ec + 1) * 512)
                    wo_t = wos.tile([P, NCORES, 2, 512], BF16, tag="wo_t",
                                    name="wo_b")
                    for j in range(NCORES):
                        nc.sync.dma_start(wo_t[:, j, :, :], wo4[:, j, 2:4, esl])
                    # hl-outer: consume wave 2 fully before wave 3 so the
                    # last collective's latency overlaps real work
                    for hl in range(2):
                        for j in range(NCORES):
                            for tt in range(TSLICE // P):
                                nc.tensor.matmul(
                                    psos_live[ec][tt][:],
                                    a2a_sb[2 + hl][:, j, tt * P:(tt + 1) * P],
                                    wo_t[:, j, hl, :],
                                    start=False,
                                    stop=(j == NCORES - 1 and hl == 1),
                                )
                    ot = outp.tile([P, TSLICE // P, 512], F32, tag="ot",
                                   name="ot")
                    for tt in range(TSLICE // P):
                        nc.vector.tensor_copy(ot[:, tt, :], psos_live[ec][tt][:])
                    nc.sync.dma_start(out3[:, :, esl], ot[:])
                    del psos_live[ec]

                group_a(0)
                for ec in range(1, 8):
                    group_a(ec)
                    group_b(ec - 1)
                group_b(7)

    _legalize_waits(nc)
    return nc


_NC_CACHE = {}


def _get_nc(reps=1):
    if reps not in _NC_CACHE:
        _install_tile_patch()
        _NC_CACHE[reps] = _build_nc(reps)
    return _NC_CACHE[reps]


def make_in_maps(x, wq, wk, wv, wo):
    bf = ml_dtypes.bfloat16
    xf = np.asarray(x, dtype=np.float32).reshape(TOK, D).astype(bf)
    wqf = np.asarray(wq, dtype=np.float32).astype(bf)
    wkf = np.asarray(wk, dtype=np.float32).astype(bf)
    wvf = np.asarray(wv, dtype=np.float32).astype(bf)
    wof = np.asarray(wo, dtype=np.float32).astype(bf)
    identv = np.eye(P, dtype=np.float32).astype(bf)
    in_maps = []
    for c in range(NCORES):
        in_maps.append({
            "x": xf,
            "ident": identv,
            "wq": np.ascontiguousarray(wqf[:, c * DQ:(c + 1) * DQ]),
            "wk": np.ascontiguousarray(wkf[:, c * HD:(c + 1) * HD]),
            "wv": np.ascontiguousarray(wvf[:, c * HD:(c + 1) * HD]),
            "wo": wof,
        })
    return in_maps


def assemble_output(results):
    out = np.concatenate([results[c]["out"] for c in range(NCORES)], axis=0)
    return out.reshape(B, S, D)


def kernel(x, wq, wk, wv, wo):
    nc = _get_nc(reps=1)
    in_maps = make_in_maps(x, wq, wk, wv, wo)
    res = run_bass_kernel_spmd(nc, in_maps, list(range(NCORES)))
    return assemble_output(res.results)


if __name__ == "__main__":
    rng = np.random.default_rng(0)
    xv = rng.standard_normal((B, S, D), dtype=np.float32)
    wqv = (rng.standard_normal((D, NH * HD), dtype=np.float32) * 0.02)
    wkv = (rng.standard_normal((D, NKV * HD), dtype=np.float32) * 0.02)
    wvv = (rng.standard_normal((D, NKV * HD), dtype=np.float32) * 0.02)
    wov = (rng.standard_normal((NH * HD, D), dtype=np.float32) * 0.02)
    got = kernel(xv, wqv, wkv, wvv, wov)
    print("kernel output", got.shape, got.dtype)
